# revision 12
# baseline (speedup 1.0000x reference)
"""GCN message-passing kernel for 8 TRN2 NeuronCores.

out = PReLU(D^-1/2 (A+I) D^-1/2 (x @ W) + b)

Strategy (per core, nodes row-sharded 12500/core, edges partitioned by dst):
  P1  h = x @ W for own rows (bf16 x blocks as lhsT, bf16 W),
      g = dinv * h (f32), written to the AllGather input bounce (+44 zero rows).
  P2  AllGather g -> g_all [8*12544, 64] f32 in local DRAM.
  P3  For each (src-bucket b in 0..3, octet of 8 dst-strips): slotted
      dma_gather of messages (int16 bucket-local indices, 1024 idx/instr,
      4 SWDGE queues); DVE strided reduce over the J message layers;
      per-bucket partial arrays in DRAM. Self loops ride along as ordinary
      edges (src == dst).
  P4  Streaming combine: per chunk of 8 dst-strips, gather the 4 partials
      in natural dst order, DVE reduce over buckets, apply dinv scale,
      bias, PReLU, write out.

Slot convention for partials/out: dst d <-> (partition p = d // 98,
column s = d % 98), 12544 slots (12500 real + 44 dummy).
"""

import os
import numpy as np

SP = os.environ.get("K_SP", "1") == "1"
JG = int(os.environ.get("K_JG", "8"))      # j-layers per msgs tile / reduce

N = 100000
E = 3200000
IN_CH = 512
OUT_CH = 64
NCORES = 8
PER = N // NCORES            # 12500 rows per core
RPAD = 12544                 # rows contributed per core (12500 + 44 zeros)
NB = 4                       # src buckets (2 cores each)
REGION = 2 * RPAD            # 25088 rows per bucket region in g_all
ZROW = PER                   # bucket-local index of a guaranteed zero row
NSTRIP = RPAD // 128         # 98 strips of 128 dsts
OCTW = 8                     # strips per octet
NT = RPAD // 128             # 98 row tiles in P1

assert RPAD % 128 == 0


def _bf16(a):
    import ml_dtypes
    return np.asarray(a, dtype=np.float32).astype(ml_dtypes.bfloat16)


def _wrap_idx(flat):
    """int16 ucode layout: idx i -> (partition i%16, col i//16), replicated
    to all 8 gpsimd cores (16-partition groups)."""
    n = flat.shape[0]
    assert n % 16 == 0
    out = np.zeros((128, n // 16), dtype=np.int16)
    w = flat.reshape(n // 16, 16).T
    for r in range(8):
        out[r * 16:(r + 1) * 16, :] = w
    return out


def _preprocess(x, edge_index, W, b, alpha):
    """Host-side index plumbing + sharding. Returns (in_maps, meta)."""
    src_g = np.asarray(edge_index[0])
    dst_g = np.asarray(edge_index[1])
    x = np.asarray(x, dtype=np.float32)
    W = np.asarray(W, dtype=np.float32)
    b = np.asarray(b, dtype=np.float32)
    alpha = np.asarray(alpha, dtype=np.float32)

    deg = np.bincount(dst_g, minlength=N).astype(np.int64) + 1  # incl self loop

    # append self loops as ordinary edges
    loops = np.arange(N, dtype=np.int64)
    src_a = np.concatenate([src_g.astype(np.int64), loops])
    dst_a = np.concatenate([dst_g.astype(np.int64), loops])

    owner = dst_a // PER
    # bucket-region-local row index of every global node id
    s_core = np.arange(N) // PER
    g_row_local = (RPAD * (s_core % 2) + (np.arange(N) % PER)).astype(np.int64)

    per_core = []
    for c in range(NCORES):
        m = owner == c
        s = src_a[m]
        d = dst_a[m] - c * PER
        bkt = s // (2 * PER)
        loc = g_row_local[s]  # bucket-local row of src in g_all region
        per_core.append((s, d, bkt, loc))

    # per (core, dst, bucket) counts
    cnts = np.zeros((NCORES, RPAD, NB), dtype=np.int32)
    orders = []  # per core: edge order sorted by (bucket, dst)
    offs = []
    for c in range(NCORES):
        s, d, bkt, loc = per_core[c]
        key = bkt * RPAD + d
        cnt = np.bincount(key, minlength=NB * RPAD).reshape(NB, RPAD)
        cnts[c] = cnt.T
        order = np.argsort(key, kind="stable")
        off = np.zeros(NB * RPAD + 1, dtype=np.int64)
        np.cumsum(cnt.reshape(-1), out=off[1:])
        orders.append(order)
        offs.append(off)

    # per-bucket dst ordering (descending count), shared strip/octet structure
    sigma = np.zeros((NCORES, NB, RPAD), dtype=np.int64)
    for c in range(NCORES):
        for bb in range(NB):
            sigma[c, bb] = np.argsort(-cnts[c, :, bb], kind="stable")

    # octets: groups of strips; unit (b, oct) has uniform J = max count over
    # ALL cores (SPMD: one program)
    octs = []  # list of (s0, w) strip ranges
    s0 = 0
    while s0 < NSTRIP:
        w = min(OCTW, NSTRIP - s0)
        octs.append((s0, w))
        s0 += w

    J_units = []
    for bb in range(NB):
        for (s0, w) in octs:
            jmax = 1
            for c in range(NCORES):
                dset = sigma[c, bb, s0 * 128:(s0 + w) * 128]
                jmax = max(jmax, int(cnts[c, dset, bb].max()))
            J_units.append(jmax)

    unit_meta = []  # (b, s0, w, J, col_off)
    col_off = 0
    ui = 0
    for bb in range(NB):
        for (s0, w) in octs:
            J = J_units[ui]
            ui += 1
            ncols = (128 * w * J) // 16
            unit_meta.append((bb, s0, w, J, col_off))
            col_off += ncols
    total_cols = col_off

    gidx = np.zeros((NCORES, 128, total_cols), dtype=np.int16)
    pad_slots_total = 0
    for c in range(NCORES):
        s, d, bkt, loc = per_core[c]
        order = orders[c]
        off = offs[c]
        loc_sorted = loc[order]
        for (bb, s0, w, J, co) in unit_meta:
            dset = sigma[c, bb, s0 * 128:(s0 + w) * 128]  # [w*128] dst ids
            cnt_u = cnts[c, dset, bb]  # [w*128]
            base = off[bb * RPAD + dset]  # [w*128] offsets into loc_sorted
            jj = np.arange(J)[:, None]  # [J, 1]
            valid = jj < cnt_u[None, :]  # [J, w*128]
            eidx = base[None, :] + np.minimum(jj, np.maximum(cnt_u[None, :] - 1, 0))
            eidx = np.minimum(eidx, loc_sorted.shape[0] - 1)
            vals = np.where(valid, loc_sorted[eidx], ZROW).astype(np.int16)
            flat = vals.reshape(J * w * 128)
            pad_slots_total += int((~valid).sum())
            gidx[c, :, co:co + flat.size // 16] = _wrap_idx(flat)

    # P4 combine index tables: 4 gathers x 12544 idx (per-bucket partials)
    # gather writes slot i to (i%128, i//128) -> idx position i corresponds
    # to dst (i%128)*98+(i//128)
    ii = np.arange(RPAD)
    dd = (ii % 128) * NSTRIP + (ii // 128)  # dst id at idx position i
    cidx = np.zeros((NCORES, NB, 128, RPAD // 16), dtype=np.int16)
    for c in range(NCORES):
        for bb in range(NB):
            spos = np.empty(RPAD, dtype=np.int64)
            spos[sigma[c, bb]] = np.arange(RPAD)
            rows = (spos[dd] % 128) * NSTRIP + (spos[dd] // 128)
            cidx[c, bb] = _wrap_idx(rows.astype(np.int16))

    # x blocks, bf16, contiguous per tile: xTb[c][r][p][k*128+m]
    #   = x[c*PER + 128r + m, 128k + p]
    xTb = np.zeros((NCORES, NT, 128, 4 * 128), dtype=np.float32)
    for c in range(NCORES):
        xr = x[c * PER:(c + 1) * PER]  # [12500, 512]
        xp = np.zeros((RPAD, IN_CH), dtype=np.float32)
        xp[:PER] = xr
        blocks = xp.reshape(NT, 128, 4, 128)  # [r, m, k, p]
        xTb[c] = blocks.transpose(0, 3, 2, 1).reshape(NT, 128, 4 * 128)
    xTb = _bf16(xTb)

    # W packed [p, k*64+c] = W[128k+p, c]
    Wb = _bf16(W.reshape(4, 128, OUT_CH).transpose(1, 0, 2).reshape(128, 4 * OUT_CH))

    # degree tables
    deg_p1 = np.zeros((NCORES, 128, NT), dtype=np.float32)
    deg_p4 = np.zeros((NCORES, 128, NSTRIP), dtype=np.float32)
    for c in range(NCORES):
        degc = np.ones(RPAD, dtype=np.float32)
        degc[:PER] = deg[c * PER:(c + 1) * PER]
        deg_p1[c] = degc.reshape(NT, 128).T  # (p, r) = row 128r+p
        deg_p4[c] = degc.reshape(128, NSTRIP)  # (p, s) = d = p*98+s
    bias_rep = np.tile(b[None, :], (128, 1)).astype(np.float32)
    alpha_rep = np.tile(alpha[None, :], (128, 1)).astype(np.float32)

    in_maps = []
    for c in range(NCORES):
        in_maps.append({
            "xTb": xTb[c],
            "Wb": Wb,
            "gidx": gidx[c],
            "cidx": cidx[c],
            "deg_p1": deg_p1[c],
            "deg_p4": deg_p4[c],
            "bias_rep": bias_rep,
            "alpha_rep": alpha_rep,
        })
    meta = {"unit_meta": unit_meta, "total_cols": total_cols,
            "pad_frac": pad_slots_total / max(1, sum(
                128 * w * J for (_, _, w, J, _) in unit_meta) * NCORES)}
    return in_maps, meta


def _patch_walrus_flags():
    import concourse.bass_utils as bu
    if getattr(bu, "_scratch_patched", False):
        return
    orig = bu.get_walrus_args

    def patched(arch, tmpdir, *, dve_root=None):
        args = orig(arch, tmpdir, dve_root=dve_root)
        import os as _os
        sz = _os.environ.get("K_DMA_SCRATCH")
        if sz:
            args = [f"--dynamic-dma-scratch-size-per-partition={sz}", *args]
        return args

    bu.get_walrus_args = patched
    bu._scratch_patched = True


def _build_program(unit_meta, total_cols, fast_pointwise, alpha0):
    import concourse.bacc as bacc
    import concourse.mybir as mybir
    import concourse.tile as tile
    from concourse.masks import make_identity

    f32 = mybir.dt.float32
    f32r = mybir.dt.float32r
    bf16 = mybir.dt.bfloat16
    i16 = mybir.dt.int16

    _patch_walrus_flags()
    nc = bacc.Bacc("TRN2", target_bir_lowering=False, debug=False,
                   num_devices=NCORES, num_swdge_queues=4,
                   dynamic_dma_scratch_size=int(os.environ.get("K_SCRATCH", "16384")))

    xTb_t = nc.dram_tensor("xTb", [NT, 128, 4 * 128], bf16, kind="ExternalInput")
    Wb_t = nc.dram_tensor("Wb", [128, 4 * OUT_CH], bf16, kind="ExternalInput")
    gidx_t = nc.dram_tensor("gidx", [128, total_cols], i16, kind="ExternalInput")
    cidx_t = nc.dram_tensor("cidx", [NB, 128, RPAD // 16], i16,
                            kind="ExternalInput")
    degp1_t = nc.dram_tensor("deg_p1", [128, NT], f32, kind="ExternalInput")
    degp4_t = nc.dram_tensor("deg_p4", [128, NSTRIP], f32, kind="ExternalInput")
    bias_t = nc.dram_tensor("bias_rep", [128, OUT_CH], f32, kind="ExternalInput")
    alpha_t = nc.dram_tensor("alpha_rep", [128, OUT_CH], f32,
                             kind="ExternalInput")
    out_t = nc.dram_tensor("out", [RPAD, OUT_CH], f32, kind="ExternalOutput")

    qc = [0]  # SWDGE queue rotation

    def nextq():
        qc[0] += 1
        return qc[0] % 4

    with tile.TileContext(nc) as tc:
        with (
            tc.tile_pool(name="dram", bufs=1, space="DRAM") as dram,
            tc.tile_pool(name="const", bufs=1) as cpool,
            tc.tile_pool(name="p1", bufs=3) as p1pool,
            tc.tile_pool(name="gath", bufs=3) as gpool,
            tc.tile_pool(name="evac", bufs=3) as epool,
            tc.tile_pool(name="p4", bufs=2) as p4pool,
            tc.tile_pool(name="psum_h", bufs=2, space="PSUM") as ppool_h,
            tc.tile_pool(name="psum_a", bufs=3, space="PSUM") as ppool_a,
            tc.tile_pool(name="psum_c", bufs=2, space="PSUM") as ppool_c,
        ):
            g_in = dram.tile([RPAD, OUT_CH], f32r)
            g_all = dram.tile([NCORES * RPAD, OUT_CH], f32r)
            partials = [dram.tile([RPAD, OUT_CH], f32r, tag=f"part{_b}",
                                  name=f"part{_b}") for _b in range(NB)]

            # ---------------- P1: h = x @ W, g = dinv * h -----------------
            Wsb = cpool.tile([128, 4, OUT_CH], bf16)
            nc.sync.dma_start(Wsb[:], Wb_t[:].rearrange("p (k c) -> p k c", k=4))
            degp1 = cpool.tile([128, NT], f32)
            nc.sync.dma_start(degp1[:], degp1_t[:])
            dinv1 = cpool.tile([128, NT], f32)
            nc.scalar.activation(dinv1[:], degp1[:],
                                 mybir.ActivationFunctionType.Sqrt)
            nc.vector.reciprocal(dinv1[:], dinv1[:])

            for r in range(NT):
                xt = p1pool.tile([128, 4, 128], bf16)
                nc.sync.dma_start(
                    xt[:], xTb_t[r].rearrange("p (k m) -> p k m", k=4))
                hp = ppool_h.tile([128, OUT_CH], f32, space="PSUM", tag="hp")
                for k in range(4):
                    nc.tensor.matmul(hp[:], xt[:, k], Wsb[:, k],
                                     start=(k == 0), stop=(k == 3))
                grow = p1pool.tile([128, OUT_CH], f32r, tag="grow")
                nc.vector.tensor_tensor(
                    out=grow[:], in0=hp[:],
                    in1=dinv1[:, r:r + 1].to_broadcast([128, OUT_CH]),
                    op=mybir.AluOpType.mult)
                nc.sync.dma_start(g_in[r * 128:(r + 1) * 128, :], grow[:])

            # ---------------- P2: AllGather ------------------------------
            nc.gpsimd.collective_compute(
                "AllGather", mybir.AluOpType.bypass,
                replica_groups=[list(range(NCORES))],
                ins=[g_in.opt()], outs=[g_all.opt()],
            )

            # ---------------- P3: gather + f32r identity matmul reduce ---
            ident_raw = cpool.tile([128, 128], f32)
            make_identity(nc, ident_raw[:])
            ident_f = cpool.tile([128, 128], f32r)
            nc.scalar.activation(ident_f[:], ident_raw[:],
                                 mybir.ActivationFunctionType.Copy)

            for (bb, s0, w, J, co) in unit_meta:
                nidx = 128 * w
                ncols_j = nidx // 16
                idx_sb = gpool.tile([128, J, ncols_j], i16, tag="idx")
                nc.sync.dma_start(
                    idx_sb[:],
                    gidx_t[:, co:co + J * ncols_j].rearrange(
                        "p (j n) -> p j n", j=J))
                acc = ppool_a.tile([128, w * OUT_CH], f32, space="PSUM",
                                   tag="acc")
                for jg in range(0, J, JG):
                    jgn = min(JG, J - jg)
                    msgs = gpool.tile([128, JG, w, OUT_CH], f32r, tag="msgs")
                    for j0 in range(0, jgn):
                        nc.gpsimd.dma_gather(
                            msgs[:, j0:j0 + 1].rearrange(
                                "p j t c -> p (j t) c"),
                            g_all[REGION * bb:REGION * (bb + 1), :],
                            idx_sb[:, jg + j0:jg + j0 + 1].rearrange(
                                "p j n -> p (j n)"),
                            nidx, nidx, OUT_CH,
                            single_packet=SP, queue_num=nextq(),
                        )
                    for j0 in range(jgn):
                        j = jg + j0
                        nc.tensor.matmul(
                            acc[:], ident_f[:],
                            msgs[:, j0].rearrange(
                                "p t c -> p (t c)"),
                            start=(j == 0), stop=(j == J - 1))
                ev = epool.tile([128, w * OUT_CH], f32r, tag="ev")
                nc.vector.tensor_copy(ev[:], acc[:])
                nc.sync.dma_start(
                    partials[bb][:].rearrange(
                        "(p s) c -> p s c", p=128)[:, s0:s0 + w, :],
                    ev[:].rearrange("p (t c) -> p t c", c=OUT_CH))

            # ---------------- P4: streaming combine + pointwise ----------
            cidx_sbs = []
            for _g in range(NB):
                cs = cpool.tile([128, RPAD // 16], i16, name=f"cidx{_g}",
                                tag=f"cidx{_g}")
                nc.sync.dma_start(cs[:], cidx_t[_g])
                cidx_sbs.append(cs)
            degp4 = cpool.tile([128, NSTRIP], f32)
            nc.sync.dma_start(degp4[:], degp4_t[:])
            dinv4 = cpool.tile([128, NSTRIP], f32)
            nc.scalar.activation(dinv4[:], degp4[:],
                                 mybir.ActivationFunctionType.Sqrt)
            nc.vector.reciprocal(dinv4[:], dinv4[:])
            bias_sb = cpool.tile([128, OUT_CH], f32)
            nc.sync.dma_start(bias_sb[:], bias_t[:])
            alpha_sb = cpool.tile([128, OUT_CH], f32)
            nc.sync.dma_start(alpha_sb[:], alpha_t[:])

            for k in range(0, NSTRIP, 8):
                kw = min(8, NSTRIP - k)
                m4 = p4pool.tile([128, NB, 8, OUT_CH], f32r, tag="m4")
                for bb in range(NB):
                    nc.gpsimd.dma_gather(
                        m4[:, bb, 0:kw], partials[bb][:],
                        cidx_sbs[bb][:, 8 * k:8 * (k + kw)],
                        128 * kw, 128 * kw, OUT_CH,
                        single_packet=SP, queue_num=nextq())
                cacc = ppool_c.tile([128, kw * OUT_CH], f32, space="PSUM",
                                    tag="cacc")
                for bb in range(NB):
                    nc.tensor.matmul(
                        cacc[:], ident_f[:],
                        m4[:, bb, 0:kw].rearrange(
                            "p t c -> p (t c)"),
                        start=(bb == 0), stop=(bb == NB - 1))
                zc = p4pool.tile([128, 8, OUT_CH], f32, tag="zc")
                if fast_pointwise:
                    # z = cacc * dinv4; out = max(z,0) + alpha0 * min(z,0)
                    nc.vector.tensor_tensor(
                        out=zc[:, 0:kw],
                        in0=cacc[:].rearrange("p (t c) -> p t c", c=OUT_CH),
                        in1=dinv4[:, k:k + kw].rearrange(
                            "p (s c) -> p s c", c=1).to_broadcast(
                                [128, kw, OUT_CH]),
                        op=mybir.AluOpType.mult)
                    rl = p4pool.tile([128, 8, OUT_CH], f32, tag="rl")
                    nc.vector.tensor_scalar(
                        out=rl[:, 0:kw], in0=zc[:, 0:kw],
                        scalar1=0.0, scalar2=None,
                        op0=mybir.AluOpType.max)
                    nc.vector.tensor_scalar(
                        out=zc[:, 0:kw], in0=zc[:, 0:kw],
                        scalar1=0.0, scalar2=alpha0,
                        op0=mybir.AluOpType.min,
                        op1=mybir.AluOpType.mult)
                    nc.vector.tensor_add(zc[:, 0:kw], zc[:, 0:kw],
                                         rl[:, 0:kw])
                else:
                    nc.vector.tensor_tensor(
                        out=zc[:, 0:kw],
                        in0=cacc[:].rearrange("p (t c) -> p t c", c=OUT_CH),
                        in1=dinv4[:, k:k + kw].rearrange(
                            "p (s c) -> p s c", c=1).to_broadcast(
                                [128, kw, OUT_CH]),
                        op=mybir.AluOpType.mult)
                    nc.vector.tensor_tensor(
                        out=zc[:, 0:kw], in0=zc[:, 0:kw],
                        in1=bias_sb[:].rearrange(
                            "p (s c) -> p s c", s=1).to_broadcast(
                                [128, kw, OUT_CH]),
                        op=mybir.AluOpType.add)
                    rl = p4pool.tile([128, 8, OUT_CH], f32, tag="rl")
                    nc.scalar.activation(rl[:, 0:kw], zc[:, 0:kw],
                                         mybir.ActivationFunctionType.Relu)
                    nc.vector.tensor_tensor(out=zc[:, 0:kw], in0=zc[:, 0:kw],
                                            in1=rl[:, 0:kw],
                                            op=mybir.AluOpType.subtract)
                    nc.vector.tensor_tensor(
                        out=zc[:, 0:kw], in0=zc[:, 0:kw],
                        in1=alpha_sb[:].rearrange(
                            "p (s c) -> p s c", s=1).to_broadcast(
                                [128, kw, OUT_CH]),
                        op=mybir.AluOpType.mult)
                    nc.vector.tensor_add(zc[:, 0:kw], zc[:, 0:kw],
                                         rl[:, 0:kw])
                nc.sync.dma_start(
                    out_t[:].rearrange("(p s) c -> p s c", p=128)[:, k:k + kw, :],
                    zc[:, 0:kw])

    nc.compile()
    return nc


LAST_EXEC_NS = None
TRACE = False


def kernel(x, edge_index, W, b, alpha):
    global LAST_EXEC_NS
    from concourse.bass_utils import run_bass_kernel_spmd

    in_maps, meta = _preprocess(x, edge_index, W, b, alpha)
    print("pad_frac:", meta["pad_frac"], "total_cols:", meta["total_cols"])
    b_arr = np.asarray(b, dtype=np.float32)
    a_arr = np.asarray(alpha, dtype=np.float32)
    fast_pw = (not np.any(b_arr)) and np.all(a_arr == a_arr[0])
    nc = _build_program(meta["unit_meta"], meta["total_cols"],
                        fast_pw, float(a_arr[0]))
    res = run_bass_kernel_spmd(nc, in_maps, core_ids=list(range(NCORES)),
                               trace=TRACE)
    LAST_EXEC_NS = res.exec_time_ns
    if TRACE:
        print("scope times:", res.per_core_scope_times)
        print("trace:", res.instructions_and_trace[1]
              if res.instructions_and_trace else None)
    out = np.concatenate([r["out"][:PER] for r in res.results], axis=0)
    return out


if __name__ == "__main__":
    pass


# revision 13
# speedup vs baseline: 1.2249x; 1.2249x over previous
"""GCN message-passing kernel for 8 TRN2 NeuronCores.

out = PReLU(D^-1/2 (A+I) D^-1/2 (x @ W) + b)

Strategy (per core, nodes row-sharded 12500/core, edges partitioned by dst):
  P1  h = x @ W for own rows (host-pre-transposed x blocks as lhsT),
      g = dinv * h, written to the AllGather input bounce (+44 zero rows).
  P2  AllGather g -> g_all [8*12544, 64] f32 in local DRAM.
  P3  For each (src-bucket b in 0..3, octet of 8 dst-strips): slotted
      dma_gather of messages (int16 bucket-local indices, 1024 idx/instr,
      4 SWDGE queues) + identity-lhsT matmuls accumulating the segmented
      sum in PSUM; evacuate to per-bucket partial arrays in DRAM.
  P4  Combine: gather the 4 partials + own g rows (self loops) into
      natural dst order, apply dinv scale, bias, PReLU, write out.

Slot convention for partials/out: dst d <-> (partition p = d // 98,
column s = d % 98), 12544 slots (12500 real + 44 dummy).
"""

import os
import numpy as np

JCAP = 8
SP = os.environ.get("K_SP", "1") == "1"
JCHUNK = int(os.environ.get("K_JCHUNK", "1"))

N = 100000
E = 3200000
IN_CH = 512
OUT_CH = 64
NCORES = 8
PER = N // NCORES            # 12500 rows per core
RPAD = 12544                 # rows contributed per core (12500 + 44 zeros)
NB = 4                       # src buckets (2 cores each)
REGION = 2 * RPAD            # 25088 rows per bucket region in g_all
ZROW = PER                   # bucket-local index of a guaranteed zero row
NSTRIP = RPAD // 128         # 98 strips of 128 dsts
OCTW = 8                     # strips per octet
NTILE = PER // 128           # 97.65... -> careful: 12500/128 not integer!

assert RPAD % 128 == 0


def _wrap_idx(flat):
    """int16 ucode layout: idx i -> (partition i%16, col i//16), replicated
    to all 8 gpsimd cores (16-partition groups)."""
    n = flat.shape[0]
    assert n % 16 == 0
    out = np.zeros((128, n // 16), dtype=np.int16)
    w = flat.reshape(n // 16, 16).T
    for r in range(8):
        out[r * 16:(r + 1) * 16, :] = w
    return out


def _preprocess(x, edge_index, W, b, alpha):
    """Host-side index plumbing + sharding. Returns (in_maps, meta)."""
    src_g = np.asarray(edge_index[0])
    dst_g = np.asarray(edge_index[1])
    x = np.asarray(x, dtype=np.float32)
    W = np.asarray(W, dtype=np.float32)
    b = np.asarray(b, dtype=np.float32)
    alpha = np.asarray(alpha, dtype=np.float32)

    deg = np.bincount(dst_g, minlength=N).astype(np.int64) + 1  # incl self loop

    owner = dst_g // PER
    # bucket-region-local row index of every global node id
    s_core = np.arange(N) // PER
    g_row_local = (RPAD * (s_core % 2) + (np.arange(N) % PER)).astype(np.int64)

    per_core = []
    for c in range(NCORES):
        m = owner == c
        s = src_g[m].astype(np.int64)
        d = (dst_g[m] - c * PER).astype(np.int64)
        bkt = s // (2 * PER)
        loc = g_row_local[s]  # bucket-local row of src in g_all region
        per_core.append((s, d, bkt, loc))

    # per (core, dst, bucket) counts
    cnts = np.zeros((NCORES, RPAD, NB), dtype=np.int32)
    orders = []  # per core: edge order sorted by (bucket, dst)
    offs = []
    for c in range(NCORES):
        s, d, bkt, loc = per_core[c]
        key = bkt * RPAD + d
        cnt = np.bincount(key, minlength=NB * RPAD).reshape(NB, RPAD)
        cnts[c] = cnt.T
        order = np.argsort(key, kind="stable")
        off = np.zeros(NB * RPAD + 1, dtype=np.int64)
        np.cumsum(cnt.reshape(-1), out=off[1:])
        orders.append(order)
        offs.append(off)

    # per-bucket dst ordering (descending count), shared strip/octet structure
    sigma = np.zeros((NCORES, NB, RPAD), dtype=np.int64)
    for c in range(NCORES):
        for bb in range(NB):
            sigma[c, bb] = np.argsort(-cnts[c, :, bb], kind="stable")

    # octets: groups of strips; unit (b, oct) has uniform J = max count over
    # ALL cores (SPMD: one program)
    octs = []  # list of (s0, w) strip ranges
    s0 = 0
    while s0 < NSTRIP:
        w = min(OCTW, NSTRIP - s0)
        octs.append((s0, w))
        s0 += w

    J_units = []
    for bb in range(NB):
        for (s0, w) in octs:
            jmax = 1
            for c in range(NCORES):
                dset = sigma[c, bb, s0 * 128:(s0 + w) * 128]
                jmax = max(jmax, int(cnts[c, dset, bb].max()))
            J_units.append(jmax)

    # build gather idx tables per core: one big int16 tensor per core
    # [128, total_cols] where unit (b,oct) occupies cols [unit_off, +64*J*? ]
    # each gather j covers 128*w*? slots... slots per j = 128*w*16? cols=slots/16
    unit_meta = []  # (b, s0, w, J, col_off)
    col_off = 0
    ui = 0
    for bb in range(NB):
        for (s0, w) in octs:
            J = J_units[ui]
            ui += 1
            ncols = (128 * w * J) // 16
            unit_meta.append((bb, s0, w, J, col_off))
            col_off += ncols
    total_cols = col_off

    gidx = np.zeros((NCORES, 128, total_cols), dtype=np.int16)
    pad_slots_total = 0
    for c in range(NCORES):
        s, d, bkt, loc = per_core[c]
        order = orders[c]
        off = offs[c]
        loc_sorted = loc[order]
        for (bb, s0, w, J, co) in unit_meta:
            dset = sigma[c, bb, s0 * 128:(s0 + w) * 128]  # [w*128] dst ids
            cnt_u = cnts[c, dset, bb]  # [w*128]
            base = off[bb * RPAD + dset]  # [w*128] offsets into loc_sorted
            # slots: for j in range(J): for t in range(w): for p in range(128)
            jj = np.arange(J)[:, None]  # [J, 1]
            valid = jj < cnt_u[None, :]  # [J, w*128]
            eidx = base[None, :] + np.minimum(jj, np.maximum(cnt_u[None, :] - 1, 0))
            eidx = np.minimum(eidx, loc_sorted.shape[0] - 1)
            vals = np.where(valid, loc_sorted[eidx], ZROW).astype(np.int16)
            # order within unit: j-major, then slot i = t*128+p
            flat = vals.reshape(J * w * 128)
            pad_slots_total += int((~valid).sum())
            gidx[c, :, co:co + flat.size // 16] = _wrap_idx(flat)

    # P4 combine index tables: 5 gathers x 12544 idx
    # out slot i -> (p=i%128, col=i//128); dst d(i) = ?  we need out tile
    # [128, 98, 64] with (P, S) = dst P*98+S. gather writes slot i to
    # (i%128, i//128) -> so idx position i corresponds to dst (i%128)*98+(i//128)
    ii = np.arange(RPAD)
    dd = (ii % 128) * NSTRIP + (ii // 128)  # dst id at idx position i
    cidx = np.zeros((NCORES, 5, 128, RPAD // 16), dtype=np.int16)
    for c in range(NCORES):
        for bb in range(NB):
            # dst d -> sigma position -> partial_b flat row (p*98 + s)
            spos = np.empty(RPAD, dtype=np.int64)
            spos[sigma[c, bb]] = np.arange(RPAD)
            rows = (spos[dd] % 128) * NSTRIP + (spos[dd] // 128)
            cidx[c, bb] = _wrap_idx(rows.astype(np.int16))
        # self-loop: own g rows, bucket-local within region c//2
        rows_own = np.where(dd < PER, dd, ZROW)
        cidx[c, 4] = _wrap_idx(rows_own.astype(np.int16))

    # x blocks: xTb[r] = x[128r:128r+128].T as 4 chunks [4,128,128]
    # rows beyond 12500 zero-padded (RPAD rows total)
    NT = RPAD // 128  # 98 row tiles
    xTb = np.zeros((NCORES, NT, 4, 128, 128), dtype=np.float32)
    for c in range(NCORES):
        xr = x[c * PER:(c + 1) * PER]  # [12500, 512]
        xp = np.zeros((RPAD, IN_CH), dtype=np.float32)
        xp[:PER] = xr
        blocks = xp.reshape(NT, 128, 4, 128)  # [r, p, k, kk]
        xTb[c] = blocks.transpose(0, 2, 3, 1)  # [r, k, kk(=K part), p(=M)]

    Wb = W.reshape(4, 128, OUT_CH).copy()

    # degree tables
    deg_p1 = np.zeros((NCORES, 128, NT), dtype=np.float32)
    deg_p4 = np.zeros((NCORES, 128, NSTRIP), dtype=np.float32)
    for c in range(NCORES):
        degc = np.ones(RPAD, dtype=np.float32)
        degc[:PER] = deg[c * PER:(c + 1) * PER]
        deg_p1[c] = degc.reshape(NT, 128).T  # (p, r) = row 128r+p
        deg_p4[c] = degc.reshape(128, NSTRIP)  # (p, s) = d = p*98+s

    bias_rep = np.tile(b[None, :], (128, 1)).astype(np.float32)
    alpha_rep = np.tile(alpha[None, :], (128, 1)).astype(np.float32)

    in_maps = []
    for c in range(NCORES):
        in_maps.append({
            "xTb": xTb[c].reshape(NT, 4 * 128 * 128),
            "Wb": Wb.reshape(4 * 128, OUT_CH),
            "gidx": gidx[c],
            "cidx": cidx[c],
            "deg_p1": deg_p1[c],
            "deg_p4": deg_p4[c],
            "bias_rep": bias_rep,
            "alpha_rep": alpha_rep,
        })
    meta = {"unit_meta": unit_meta, "total_cols": total_cols,
            "pad_frac": pad_slots_total / max(1, sum(
                128 * w * J for (_, _, w, J, _) in unit_meta) * NCORES)}
    return in_maps, meta


def _patch_walrus_flags():
    import concourse.bass_utils as bu
    if getattr(bu, "_scratch_patched", False):
        return
    orig = bu.get_walrus_args

    def patched(arch, tmpdir, *, dve_root=None):
        args = orig(arch, tmpdir, dve_root=dve_root)
        import os as _os
        sz = _os.environ.get("K_DMA_SCRATCH")
        if sz:
            args = [f"--dynamic-dma-scratch-size-per-partition={sz}", *args]
        return args

    bu.get_walrus_args = patched
    bu._scratch_patched = True


def _build_program(unit_meta, total_cols):
    import concourse.bacc as bacc
    import concourse.mybir as mybir
    import concourse.tile as tile
    from concourse.masks import make_identity

    f32 = mybir.dt.float32
    f32r = mybir.dt.float32r
    i16 = mybir.dt.int16
    NT = RPAD // 128

    _patch_walrus_flags()
    nc = bacc.Bacc("TRN2", target_bir_lowering=False, debug=False,
                   num_devices=NCORES, num_swdge_queues=4,
                   dynamic_dma_scratch_size=int(os.environ.get("K_SCRATCH", "16384")))

    xTb_t = nc.dram_tensor("xTb", [NT, 4 * 128 * 128], f32, kind="ExternalInput")
    Wb_t = nc.dram_tensor("Wb", [4 * 128, OUT_CH], f32, kind="ExternalInput")
    gidx_t = nc.dram_tensor("gidx", [128, total_cols], i16, kind="ExternalInput")
    cidx_t = nc.dram_tensor("cidx", [5, 128, RPAD // 16], i16, kind="ExternalInput")
    degp1_t = nc.dram_tensor("deg_p1", [128, NT], f32, kind="ExternalInput")
    degp4_t = nc.dram_tensor("deg_p4", [128, NSTRIP], f32, kind="ExternalInput")
    bias_t = nc.dram_tensor("bias_rep", [128, OUT_CH], f32, kind="ExternalInput")
    alpha_t = nc.dram_tensor("alpha_rep", [128, OUT_CH], f32, kind="ExternalInput")
    out_t = nc.dram_tensor("out", [RPAD, OUT_CH], f32, kind="ExternalOutput")

    with tile.TileContext(nc) as tc:
        with (
            tc.tile_pool(name="dram", bufs=1, space="DRAM") as dram,
            tc.tile_pool(name="const", bufs=1) as cpool,
            tc.tile_pool(name="p1", bufs=3) as p1pool,
            tc.tile_pool(name="gath", bufs=2) as gpool,
            tc.tile_pool(name="evac", bufs=3) as epool,
            tc.tile_pool(name="p4", bufs=1) as p4pool,
            tc.tile_pool(name="psum_h", bufs=2, space="PSUM") as ppool_h,
            tc.tile_pool(name="psum_a", bufs=4, space="PSUM") as ppool,
        ):
            g_in = dram.tile([RPAD, OUT_CH], f32r)
            g_all = dram.tile([NCORES * RPAD, OUT_CH], f32r)
            partials = [dram.tile([RPAD, OUT_CH], f32r, tag=f"part{_b}", name=f"part{_b}")
                        for _b in range(NB)]

            # ---------------- P1: h = x @ W, g = dinv * h -----------------
            Wsb = cpool.tile([128, 4, OUT_CH], f32)
            nc.sync.dma_start(Wsb[:], Wb_t[:].rearrange("(k p) c -> p k c", p=128))
            degp1 = cpool.tile([128, NT], f32)
            nc.sync.dma_start(degp1[:], degp1_t[:])
            dinv1 = cpool.tile([128, NT], f32)
            nc.scalar.activation(dinv1[:], degp1[:],
                                 mybir.ActivationFunctionType.Sqrt)
            nc.vector.reciprocal(dinv1[:], dinv1[:])

            for r in range(NT):
                xt = p1pool.tile([128, 4, 128], f32)
                nc.sync.dma_start(
                    xt[:], xTb_t[r].rearrange("(k p m) -> p k m", k=4, p=128))
                hp = ppool_h.tile([128, OUT_CH], f32, space="PSUM", tag="hp")
                for k in range(4):
                    nc.tensor.matmul(hp[:], xt[:, k], Wsb[:, k],
                                     start=(k == 0), stop=(k == 3))
                grow = p1pool.tile([128, OUT_CH], f32r, tag="grow")
                nc.scalar.activation(grow[:], hp[:],
                                     mybir.ActivationFunctionType.Copy,
                                     scale=dinv1[:, r:r + 1])
                nc.sync.dma_start(g_in[r * 128:(r + 1) * 128, :], grow[:])

            # ---------------- P2: AllGather ------------------------------
            nc.gpsimd.collective_compute(
                "AllGather", mybir.AluOpType.bypass,
                replica_groups=[list(range(NCORES))],
                ins=[g_in.opt()], outs=[g_all.opt()],
            )

            # ---------------- P3: gather + segmented reduce --------------
            ident_raw = cpool.tile([128, 128], f32)
            make_identity(nc, ident_raw[:])
            ident_f = cpool.tile([128, 128], f32r)
            nc.scalar.activation(ident_f[:], ident_raw[:],
                                 mybir.ActivationFunctionType.Copy)

            for (bb, s0, w, J, co) in unit_meta:
                nidx = 128 * w
                ncols_j = nidx // 16
                idx_sb = gpool.tile([128, J, ncols_j], i16, tag="idx")
                nc.sync.dma_start(
                    idx_sb[:],
                    gidx_t[:, co:co + J * ncols_j].rearrange(
                        "p (j n) -> p j n", j=J))
                acc = ppool.tile([128, w * OUT_CH], f32, space="PSUM", tag="acc")
                for jg in range(0, J, JCAP):
                    jgn = min(JCAP, J - jg)
                    msgs = gpool.tile([128, jgn, w, OUT_CH], f32r, tag="msgs")
                    for j0 in range(0, jgn, JCHUNK):
                        jn = min(JCHUNK, jgn - j0)
                        nc.gpsimd.dma_gather(
                            msgs[:, j0:j0 + jn].rearrange(
                                "p j t c -> p (j t) c"),
                            g_all[REGION * bb:REGION * (bb + 1), :],
                            idx_sb[:, jg + j0:jg + j0 + jn].rearrange(
                                "p j n -> p (j n)"),
                            nidx * jn, nidx * jn, OUT_CH,
                            single_packet=SP,
                            queue_num=((jg + j0) // JCHUNK) % 4,
                        )
                    for j in range(jgn):
                        nc.tensor.matmul(
                            acc[:], ident_f[:],
                            msgs[:, j].rearrange("p t c -> p (t c)"),
                            start=(jg + j == 0), stop=(jg + j == J - 1))
                ev = epool.tile([128, w * OUT_CH], f32r, tag="ev")
                nc.vector.tensor_copy(ev[:], acc[:])
                nc.sync.dma_start(
                    partials[bb][:].rearrange(
                        "(p s) c -> p s c", p=128)[:, s0:s0 + w, :],
                    ev[:].rearrange("p (t c) -> p t c", c=OUT_CH))

            # ---------------- P4: combine + pointwise --------------------
            cidx_sbs = []
            for _g in range(5):
                cs = cpool.tile([128, RPAD // 16], i16, name=f"cidx{_g}",
                                tag=f"cidx{_g}")
                nc.sync.dma_start(cs[:], cidx_t[_g])
                cidx_sbs.append(cs)
            degp4 = cpool.tile([128, NSTRIP], f32)
            nc.sync.dma_start(degp4[:], degp4_t[:])
            dinv4 = cpool.tile([128, NSTRIP], f32)
            nc.scalar.activation(dinv4[:], degp4[:],
                                 mybir.ActivationFunctionType.Sqrt)
            nc.vector.reciprocal(dinv4[:], dinv4[:])
            bias_sb = cpool.tile([128, OUT_CH], f32)
            nc.sync.dma_start(bias_sb[:], bias_t[:])
            alpha_sb = cpool.tile([128, OUT_CH], f32)
            nc.sync.dma_start(alpha_sb[:], alpha_t[:])

            accs = p4pool.tile([128, NSTRIP, OUT_CH], f32, tag="c_acc")
            tmp = p4pool.tile([128, NSTRIP, OUT_CH], f32, tag="c_tmp")
            for bb in range(NB):
                tgt = accs if bb == 0 else tmp
                for k in range(0, RPAD // 128, 8):
                    kw = min(8, RPAD // 128 - k)
                    nc.gpsimd.dma_gather(
                        tgt[:, k:k + kw], partials[bb][:].bitcast(f32),
                        cidx_sbs[bb][:, 8 * k:8 * k + 8 * kw],
                        128 * kw, 128 * kw, OUT_CH,
                        single_packet=SP, queue_num=(k // 8) % 4)
                if bb > 0:
                    nc.vector.tensor_add(accs[:], accs[:], tmp[:])
            for k in range(0, RPAD // 128, 8):
                kw = min(8, RPAD // 128 - k)
                nc.gpsimd.dma_gather(
                    tmp[:, k:k + kw], g_in[:].bitcast(f32),
                    cidx_sbs[4][:, 8 * k:8 * k + 8 * kw],
                    128 * kw, 128 * kw, OUT_CH,
                    single_packet=SP, queue_num=(k // 8) % 4)
            nc.vector.tensor_add(accs[:], accs[:], tmp[:])

            # z = accs * dinv4 + bias (in place in accs)
            nc.vector.tensor_tensor(
                out=accs[:], in0=accs[:],
                in1=dinv4[:].rearrange("p (s c) -> p s c", c=1).to_broadcast(
                    [128, NSTRIP, OUT_CH]),
                op=mybir.AluOpType.mult)
            nc.vector.tensor_tensor(
                out=accs[:], in0=accs[:],
                in1=bias_sb[:].rearrange("p (s c) -> p s c", s=1).to_broadcast(
                    [128, NSTRIP, OUT_CH]),
                op=mybir.AluOpType.add)
            # tmp = relu(z); accs = (z - relu(z)) * alpha + relu(z)
            nc.scalar.activation(tmp[:], accs[:],
                                 mybir.ActivationFunctionType.Relu)
            nc.vector.tensor_tensor(out=accs[:], in0=accs[:], in1=tmp[:],
                                    op=mybir.AluOpType.subtract)
            nc.vector.tensor_tensor(
                out=accs[:], in0=accs[:],
                in1=alpha_sb[:].rearrange("p (s c) -> p s c", s=1).to_broadcast(
                    [128, NSTRIP, OUT_CH]),
                op=mybir.AluOpType.mult)
            nc.vector.tensor_add(accs[:], accs[:], tmp[:])
            nc.sync.dma_start(
                out_t[:].rearrange("(p s) c -> p s c", p=128), accs[:])

    nc.compile()
    return nc


LAST_EXEC_NS = None
TRACE = False


def kernel(x, edge_index, W, b, alpha):
    global LAST_EXEC_NS
    from concourse.bass_utils import run_bass_kernel_spmd

    in_maps, meta = _preprocess(x, edge_index, W, b, alpha)
    print("pad_frac:", meta["pad_frac"], "total_cols:", meta["total_cols"])
    nc = _build_program(meta["unit_meta"], meta["total_cols"])
    res = run_bass_kernel_spmd(nc, in_maps, core_ids=list(range(NCORES)),
                               trace=TRACE)
    LAST_EXEC_NS = res.exec_time_ns
    if TRACE:
        print("scope times:", res.per_core_scope_times)
        print("trace:", res.instructions_and_trace[1]
              if res.instructions_and_trace else None)
    out = np.concatenate([r["out"][:PER] for r in res.results], axis=0)
    return out


if __name__ == "__main__":
    pass



# revision 14
# speedup vs baseline: 1.9014x; 1.5523x over previous
"""GCN message-passing kernel for 8 TRN2 NeuronCores.

out = PReLU(D^-1/2 (A+I) D^-1/2 (x @ W) + b)

Strategy (per core, nodes row-sharded 12500/core, edges partitioned by dst):
  P1  h = x @ W for own rows (host-pre-transposed x blocks as lhsT),
      g = dinv * h, written to the AllGather input bounce (+44 zero rows).
  P2  AllGather g -> g_all [8*12544, 64] f32 in local DRAM.
  P3  For each (src-bucket b in 0..3, octet of 8 dst-strips): slotted
      dma_gather of messages (int16 bucket-local indices, 1024 idx/instr,
      4 SWDGE queues) + identity-lhsT matmuls accumulating the segmented
      sum in PSUM; evacuate to per-bucket partial arrays in DRAM.
  P4  Combine: gather the 4 partials + own g rows (self loops) into
      natural dst order, apply dinv scale, bias, PReLU, write out.

Slot convention for partials/out: dst d <-> (partition p = d // 98,
column s = d % 98), 12544 slots (12500 real + 44 dummy).
"""

import os
import numpy as np

JCAP = 8
SP = os.environ.get("K_SP", "1") == "1"
JCHUNK = int(os.environ.get("K_JCHUNK", "1"))

N = 100000
E = 3200000
IN_CH = 512
OUT_CH = 64
NCORES = 8
PER = N // NCORES            # 12500 rows per core
RPAD = 12544                 # rows contributed per core (12500 + 44 zeros)
NB = 4                       # src buckets (2 cores each)
REGION = 2 * RPAD            # 25088 rows per bucket region in g_all
ZROW = PER                   # bucket-local index of a guaranteed zero row
NSTRIP = RPAD // 128         # 98 strips of 128 dsts
OCTW = 8                     # strips per octet
NTILE = PER // 128           # 97.65... -> careful: 12500/128 not integer!

assert RPAD % 128 == 0


def _wrap_idx(flat):
    """int16 ucode layout: idx i -> (partition i%16, col i//16), replicated
    to all 8 gpsimd cores (16-partition groups)."""
    n = flat.shape[0]
    assert n % 16 == 0
    out = np.zeros((128, n // 16), dtype=np.int16)
    w = flat.reshape(n // 16, 16).T
    for r in range(8):
        out[r * 16:(r + 1) * 16, :] = w
    return out


def _preprocess(x, edge_index, W, b, alpha):
    """Host-side index plumbing + sharding. Returns (in_maps, meta)."""
    src_g = np.asarray(edge_index[0])
    dst_g = np.asarray(edge_index[1])
    x = np.asarray(x, dtype=np.float32)
    W = np.asarray(W, dtype=np.float32)
    b = np.asarray(b, dtype=np.float32)
    alpha = np.asarray(alpha, dtype=np.float32)

    deg = np.bincount(dst_g, minlength=N).astype(np.int64) + 1  # incl self loop

    owner = dst_g // PER
    # bucket-region-local row index of every global node id
    s_core = np.arange(N) // PER
    g_row_local = (RPAD * (s_core % 2) + (np.arange(N) % PER)).astype(np.int64)

    per_core = []
    for c in range(NCORES):
        m = owner == c
        s = src_g[m].astype(np.int64)
        d = (dst_g[m] - c * PER).astype(np.int64)
        bkt = s // (2 * PER)
        loc = g_row_local[s]  # bucket-local row of src in g_all region
        per_core.append((s, d, bkt, loc))

    # per (core, dst, bucket) counts
    cnts = np.zeros((NCORES, RPAD, NB), dtype=np.int32)
    orders = []  # per core: edge order sorted by (bucket, dst)
    offs = []
    for c in range(NCORES):
        s, d, bkt, loc = per_core[c]
        key = bkt * RPAD + d
        cnt = np.bincount(key, minlength=NB * RPAD).reshape(NB, RPAD)
        cnts[c] = cnt.T
        order = np.argsort(key, kind="stable")
        off = np.zeros(NB * RPAD + 1, dtype=np.int64)
        np.cumsum(cnt.reshape(-1), out=off[1:])
        orders.append(order)
        offs.append(off)

    # per-bucket dst ordering (descending count), shared strip/octet structure
    sigma = np.zeros((NCORES, NB, RPAD), dtype=np.int64)
    for c in range(NCORES):
        for bb in range(NB):
            sigma[c, bb] = np.argsort(-cnts[c, :, bb], kind="stable")

    # octets: groups of strips; unit (b, oct) has uniform J = max count over
    # ALL cores (SPMD: one program)
    octs = []  # list of (s0, w) strip ranges
    s0 = 0
    while s0 < NSTRIP:
        w = min(OCTW, NSTRIP - s0)
        octs.append((s0, w))
        s0 += w

    J_units = []
    for bb in range(NB):
        for (s0, w) in octs:
            jmax = 1
            for c in range(NCORES):
                dset = sigma[c, bb, s0 * 128:(s0 + w) * 128]
                jmax = max(jmax, int(cnts[c, dset, bb].max()))
            J_units.append(jmax)

    # build gather idx tables per core: one big int16 tensor per core
    # [128, total_cols] where unit (b,oct) occupies cols [unit_off, +64*J*? ]
    # each gather j covers 128*w*? slots... slots per j = 128*w*16? cols=slots/16
    unit_meta = []  # (b, s0, w, J, col_off)
    col_off = 0
    ui = 0
    for bb in range(NB):
        for (s0, w) in octs:
            J = J_units[ui]
            ui += 1
            ncols = (128 * w * J) // 16
            unit_meta.append((bb, s0, w, J, col_off))
            col_off += ncols
    total_cols = col_off

    gidx = np.zeros((NCORES, 128, total_cols), dtype=np.int16)
    pad_slots_total = 0
    for c in range(NCORES):
        s, d, bkt, loc = per_core[c]
        order = orders[c]
        off = offs[c]
        loc_sorted = loc[order]
        for (bb, s0, w, J, co) in unit_meta:
            dset = sigma[c, bb, s0 * 128:(s0 + w) * 128]  # [w*128] dst ids
            cnt_u = cnts[c, dset, bb]  # [w*128]
            base = off[bb * RPAD + dset]  # [w*128] offsets into loc_sorted
            # slots: for j in range(J): for t in range(w): for p in range(128)
            jj = np.arange(J)[:, None]  # [J, 1]
            valid = jj < cnt_u[None, :]  # [J, w*128]
            eidx = base[None, :] + np.minimum(jj, np.maximum(cnt_u[None, :] - 1, 0))
            eidx = np.minimum(eidx, loc_sorted.shape[0] - 1)
            # Padded slots read a zero row. Spread them over the 2*44
            # distinct zero rows of the region (rows PER..RPAD of both
            # core blocks): identical-address reads serialize the SDMA
            # read pipeline (~400ns/desc instead of ~25ns).
            pos = np.arange(J * w * 128).reshape(J, w * 128)
            zvar = (ZROW + pos % 44
                    + RPAD * ((pos // 44) % 2)).astype(np.int64)
            vals = np.where(valid, loc_sorted[eidx], zvar).astype(np.int16)
            # order within unit: j-major, then slot i = t*128+p
            flat = vals.reshape(J * w * 128)
            pad_slots_total += int((~valid).sum())
            gidx[c, :, co:co + flat.size // 16] = _wrap_idx(flat)

    # P4 combine index tables: 5 gathers x 12544 idx
    # out slot i -> (p=i%128, col=i//128); dst d(i) = ?  we need out tile
    # [128, 98, 64] with (P, S) = dst P*98+S. gather writes slot i to
    # (i%128, i//128) -> so idx position i corresponds to dst (i%128)*98+(i//128)
    ii = np.arange(RPAD)
    dd = (ii % 128) * NSTRIP + (ii // 128)  # dst id at idx position i
    cidx = np.zeros((NCORES, 5, 128, RPAD // 16), dtype=np.int16)
    for c in range(NCORES):
        for bb in range(NB):
            # dst d -> sigma position -> partial_b flat row (p*98 + s)
            spos = np.empty(RPAD, dtype=np.int64)
            spos[sigma[c, bb]] = np.arange(RPAD)
            rows = (spos[dd] % 128) * NSTRIP + (spos[dd] // 128)
            cidx[c, bb] = _wrap_idx(rows.astype(np.int16))
        # self-loop: own g rows, bucket-local within region c//2
        rows_own = np.where(dd < PER, dd, ZROW)
        cidx[c, 4] = _wrap_idx(rows_own.astype(np.int16))

    # x blocks: xTb[r] = x[128r:128r+128].T as 4 chunks [4,128,128]
    # rows beyond 12500 zero-padded (RPAD rows total)
    NT = RPAD // 128  # 98 row tiles
    xTb = np.zeros((NCORES, NT, 4, 128, 128), dtype=np.float32)
    for c in range(NCORES):
        xr = x[c * PER:(c + 1) * PER]  # [12500, 512]
        xp = np.zeros((RPAD, IN_CH), dtype=np.float32)
        xp[:PER] = xr
        blocks = xp.reshape(NT, 128, 4, 128)  # [r, p, k, kk]
        xTb[c] = blocks.transpose(0, 2, 3, 1)  # [r, k, kk(=K part), p(=M)]

    Wb = W.reshape(4, 128, OUT_CH).copy()

    # degree tables
    deg_p1 = np.zeros((NCORES, 128, NT), dtype=np.float32)
    deg_p4 = np.zeros((NCORES, 128, NSTRIP), dtype=np.float32)
    for c in range(NCORES):
        degc = np.ones(RPAD, dtype=np.float32)
        degc[:PER] = deg[c * PER:(c + 1) * PER]
        deg_p1[c] = degc.reshape(NT, 128).T  # (p, r) = row 128r+p
        deg_p4[c] = degc.reshape(128, NSTRIP)  # (p, s) = d = p*98+s

    bias_rep = np.tile(b[None, :], (128, 1)).astype(np.float32)
    alpha_rep = np.tile(alpha[None, :], (128, 1)).astype(np.float32)

    in_maps = []
    for c in range(NCORES):
        in_maps.append({
            "xTb": xTb[c].reshape(NT, 4 * 128 * 128),
            "Wb": Wb.reshape(4 * 128, OUT_CH),
            "gidx": gidx[c],
            "cidx": cidx[c],
            "deg_p1": deg_p1[c],
            "deg_p4": deg_p4[c],
            "bias_rep": bias_rep,
            "alpha_rep": alpha_rep,
        })
    meta = {"unit_meta": unit_meta, "total_cols": total_cols,
            "pad_frac": pad_slots_total / max(1, sum(
                128 * w * J for (_, _, w, J, _) in unit_meta) * NCORES)}
    return in_maps, meta


def _patch_walrus_flags():
    import concourse.bass_utils as bu
    if getattr(bu, "_scratch_patched", False):
        return
    orig = bu.get_walrus_args

    def patched(arch, tmpdir, *, dve_root=None):
        args = orig(arch, tmpdir, dve_root=dve_root)
        import os as _os
        sz = _os.environ.get("K_DMA_SCRATCH")
        if sz:
            args = [f"--dynamic-dma-scratch-size-per-partition={sz}", *args]
        return args

    bu.get_walrus_args = patched
    bu._scratch_patched = True


def _build_program(unit_meta, total_cols):
    import concourse.bacc as bacc
    import concourse.mybir as mybir
    import concourse.tile as tile
    from concourse.masks import make_identity

    f32 = mybir.dt.float32
    f32r = mybir.dt.float32r
    i16 = mybir.dt.int16
    NT = RPAD // 128

    _patch_walrus_flags()
    nc = bacc.Bacc("TRN2", target_bir_lowering=False, debug=False,
                   num_devices=NCORES, num_swdge_queues=4,
                   dynamic_dma_scratch_size=int(os.environ.get("K_SCRATCH", "16384")))

    xTb_t = nc.dram_tensor("xTb", [NT, 4 * 128 * 128], f32, kind="ExternalInput")
    Wb_t = nc.dram_tensor("Wb", [4 * 128, OUT_CH], f32, kind="ExternalInput")
    gidx_t = nc.dram_tensor("gidx", [128, total_cols], i16, kind="ExternalInput")
    cidx_t = nc.dram_tensor("cidx", [5, 128, RPAD // 16], i16, kind="ExternalInput")
    degp1_t = nc.dram_tensor("deg_p1", [128, NT], f32, kind="ExternalInput")
    degp4_t = nc.dram_tensor("deg_p4", [128, NSTRIP], f32, kind="ExternalInput")
    bias_t = nc.dram_tensor("bias_rep", [128, OUT_CH], f32, kind="ExternalInput")
    alpha_t = nc.dram_tensor("alpha_rep", [128, OUT_CH], f32, kind="ExternalInput")
    out_t = nc.dram_tensor("out", [RPAD, OUT_CH], f32, kind="ExternalOutput")

    with tile.TileContext(nc) as tc:
        with (
            tc.tile_pool(name="dram", bufs=1, space="DRAM") as dram,
            tc.tile_pool(name="const", bufs=1) as cpool,
            tc.tile_pool(name="p1", bufs=3) as p1pool,
            tc.tile_pool(name="gath", bufs=2) as gpool,
            tc.tile_pool(name="evac", bufs=3) as epool,
            tc.tile_pool(name="p4", bufs=1) as p4pool,
            tc.tile_pool(name="psum_h", bufs=2, space="PSUM") as ppool_h,
            tc.tile_pool(name="psum_a", bufs=4, space="PSUM") as ppool,
        ):
            g_in = dram.tile([RPAD, OUT_CH], f32r)
            g_all = dram.tile([NCORES * RPAD, OUT_CH], f32r)
            partials = [dram.tile([RPAD, OUT_CH], f32r, tag=f"part{_b}", name=f"part{_b}")
                        for _b in range(NB)]

            # ---------------- P1: h = x @ W, g = dinv * h -----------------
            Wsb = cpool.tile([128, 4, OUT_CH], f32)
            nc.sync.dma_start(Wsb[:], Wb_t[:].rearrange("(k p) c -> p k c", p=128))
            degp1 = cpool.tile([128, NT], f32)
            nc.sync.dma_start(degp1[:], degp1_t[:])
            dinv1 = cpool.tile([128, NT], f32)
            nc.scalar.activation(dinv1[:], degp1[:],
                                 mybir.ActivationFunctionType.Sqrt)
            nc.vector.reciprocal(dinv1[:], dinv1[:])

            for r in range(NT):
                xt = p1pool.tile([128, 4, 128], f32)
                nc.sync.dma_start(
                    xt[:], xTb_t[r].rearrange("(k p m) -> p k m", k=4, p=128))
                hp = ppool_h.tile([128, OUT_CH], f32, space="PSUM", tag="hp")
                for k in range(4):
                    nc.tensor.matmul(hp[:], xt[:, k], Wsb[:, k],
                                     start=(k == 0), stop=(k == 3))
                grow = p1pool.tile([128, OUT_CH], f32r, tag="grow")
                nc.scalar.activation(grow[:], hp[:],
                                     mybir.ActivationFunctionType.Copy,
                                     scale=dinv1[:, r:r + 1])
                nc.sync.dma_start(g_in[r * 128:(r + 1) * 128, :], grow[:])

            # ---------------- P2: AllGather ------------------------------
            nc.gpsimd.collective_compute(
                "AllGather", mybir.AluOpType.bypass,
                replica_groups=[list(range(NCORES))],
                ins=[g_in.opt()], outs=[g_all.opt()],
            )

            # ---------------- P3: gather + segmented reduce --------------
            ident_raw = cpool.tile([128, 128], f32)
            make_identity(nc, ident_raw[:])
            ident_f = cpool.tile([128, 128], f32r)
            nc.scalar.activation(ident_f[:], ident_raw[:],
                                 mybir.ActivationFunctionType.Copy)

            for (bb, s0, w, J, co) in unit_meta:
                nidx = 128 * w
                ncols_j = nidx // 16
                idx_sb = gpool.tile([128, J, ncols_j], i16, tag="idx")
                nc.sync.dma_start(
                    idx_sb[:],
                    gidx_t[:, co:co + J * ncols_j].rearrange(
                        "p (j n) -> p j n", j=J))
                acc = ppool.tile([128, w * OUT_CH], f32, space="PSUM", tag="acc")
                for jg in range(0, J, JCAP):
                    jgn = min(JCAP, J - jg)
                    msgs = gpool.tile([128, jgn, w, OUT_CH], f32r, tag="msgs")
                    for j0 in range(0, jgn, JCHUNK):
                        jn = min(JCHUNK, jgn - j0)
                        nc.gpsimd.dma_gather(
                            msgs[:, j0:j0 + jn].rearrange(
                                "p j t c -> p (j t) c"),
                            g_all[REGION * bb:REGION * (bb + 1), :],
                            idx_sb[:, jg + j0:jg + j0 + jn].rearrange(
                                "p j n -> p (j n)"),
                            nidx * jn, nidx * jn, OUT_CH,
                            single_packet=SP,
                            queue_num=((jg + j0) // JCHUNK) % 4,
                        )
                    for j in range(jgn):
                        nc.tensor.matmul(
                            acc[:], ident_f[:],
                            msgs[:, j].rearrange("p t c -> p (t c)"),
                            start=(jg + j == 0), stop=(jg + j == J - 1))
                ev = epool.tile([128, w * OUT_CH], f32r, tag="ev")
                nc.vector.tensor_copy(ev[:], acc[:])
                nc.sync.dma_start(
                    partials[bb][:].rearrange(
                        "(p s) c -> p s c", p=128)[:, s0:s0 + w, :],
                    ev[:].rearrange("p (t c) -> p t c", c=OUT_CH))

            # ---------------- P4: combine + pointwise --------------------
            cidx_sbs = []
            for _g in range(5):
                cs = cpool.tile([128, RPAD // 16], i16, name=f"cidx{_g}",
                                tag=f"cidx{_g}")
                nc.sync.dma_start(cs[:], cidx_t[_g])
                cidx_sbs.append(cs)
            degp4 = cpool.tile([128, NSTRIP], f32)
            nc.sync.dma_start(degp4[:], degp4_t[:])
            dinv4 = cpool.tile([128, NSTRIP], f32)
            nc.scalar.activation(dinv4[:], degp4[:],
                                 mybir.ActivationFunctionType.Sqrt)
            nc.vector.reciprocal(dinv4[:], dinv4[:])
            bias_sb = cpool.tile([128, OUT_CH], f32)
            nc.sync.dma_start(bias_sb[:], bias_t[:])
            alpha_sb = cpool.tile([128, OUT_CH], f32)
            nc.sync.dma_start(alpha_sb[:], alpha_t[:])

            accs = p4pool.tile([128, NSTRIP, OUT_CH], f32, tag="c_acc")
            tmp = p4pool.tile([128, NSTRIP, OUT_CH], f32, tag="c_tmp")
            for bb in range(NB):
                tgt = accs if bb == 0 else tmp
                for k in range(0, RPAD // 128, 8):
                    kw = min(8, RPAD // 128 - k)
                    nc.gpsimd.dma_gather(
                        tgt[:, k:k + kw], partials[bb][:].bitcast(f32),
                        cidx_sbs[bb][:, 8 * k:8 * k + 8 * kw],
                        128 * kw, 128 * kw, OUT_CH,
                        single_packet=SP, queue_num=(k // 8) % 4)
                if bb > 0:
                    nc.vector.tensor_add(accs[:], accs[:], tmp[:])
            for k in range(0, RPAD // 128, 8):
                kw = min(8, RPAD // 128 - k)
                nc.gpsimd.dma_gather(
                    tmp[:, k:k + kw], g_in[:].bitcast(f32),
                    cidx_sbs[4][:, 8 * k:8 * k + 8 * kw],
                    128 * kw, 128 * kw, OUT_CH,
                    single_packet=SP, queue_num=(k // 8) % 4)
            nc.vector.tensor_add(accs[:], accs[:], tmp[:])

            # z = accs * dinv4 + bias (in place in accs)
            nc.vector.tensor_tensor(
                out=accs[:], in0=accs[:],
                in1=dinv4[:].rearrange("p (s c) -> p s c", c=1).to_broadcast(
                    [128, NSTRIP, OUT_CH]),
                op=mybir.AluOpType.mult)
            nc.vector.tensor_tensor(
                out=accs[:], in0=accs[:],
                in1=bias_sb[:].rearrange("p (s c) -> p s c", s=1).to_broadcast(
                    [128, NSTRIP, OUT_CH]),
                op=mybir.AluOpType.add)
            # tmp = relu(z); accs = (z - relu(z)) * alpha + relu(z)
            nc.scalar.activation(tmp[:], accs[:],
                                 mybir.ActivationFunctionType.Relu)
            nc.vector.tensor_tensor(out=accs[:], in0=accs[:], in1=tmp[:],
                                    op=mybir.AluOpType.subtract)
            nc.vector.tensor_tensor(
                out=accs[:], in0=accs[:],
                in1=alpha_sb[:].rearrange("p (s c) -> p s c", s=1).to_broadcast(
                    [128, NSTRIP, OUT_CH]),
                op=mybir.AluOpType.mult)
            nc.vector.tensor_add(accs[:], accs[:], tmp[:])
            nc.sync.dma_start(
                out_t[:].rearrange("(p s) c -> p s c", p=128), accs[:])

    nc.compile()
    return nc


LAST_EXEC_NS = None
TRACE = False


def kernel(x, edge_index, W, b, alpha):
    global LAST_EXEC_NS
    from concourse.bass_utils import run_bass_kernel_spmd

    in_maps, meta = _preprocess(x, edge_index, W, b, alpha)
    print("pad_frac:", meta["pad_frac"], "total_cols:", meta["total_cols"])
    nc = _build_program(meta["unit_meta"], meta["total_cols"])
    res = run_bass_kernel_spmd(nc, in_maps, core_ids=list(range(NCORES)),
                               trace=TRACE)
    LAST_EXEC_NS = res.exec_time_ns
    if TRACE:
        print("scope times:", res.per_core_scope_times)
        print("trace:", res.instructions_and_trace[1]
              if res.instructions_and_trace else None)
    out = np.concatenate([r["out"][:PER] for r in res.results], axis=0)
    return out


if __name__ == "__main__":
    pass



# revision 16
# speedup vs baseline: 2.0378x; 1.0717x over previous
"""GCN message-passing kernel for 8 TRN2 NeuronCores.

out = PReLU(D^-1/2 (A+I) D^-1/2 (x @ W) + b)

Strategy (per core, nodes row-sharded 12500/core, edges partitioned by dst):
  P1  h = x @ W for own rows (host-pre-transposed x blocks as lhsT),
      g = dinv * h, written to the AllGather input bounce (+44 zero rows).
  P2  AllGather g -> g_all [8*12544, 64] f32 in local DRAM.
  P3  For each (src-bucket b in 0..3, octet of 8 dst-strips): slotted
      dma_gather of messages (int16 bucket-local indices, 1024 idx/instr,
      4 SWDGE queues) + identity-lhsT matmuls accumulating the segmented
      sum in PSUM; evacuate to per-bucket partial arrays in DRAM.
  P4  Combine: gather the 4 partials + own g rows (self loops) into
      natural dst order, apply dinv scale, bias, PReLU, write out.

Slot convention for partials/out: dst d <-> (partition p = d // 98,
column s = d % 98), 12544 slots (12500 real + 44 dummy).
"""

import os
import numpy as np

JCAP = 8
SP = os.environ.get("K_SP", "1") == "1"
JCHUNK = int(os.environ.get("K_JCHUNK", "1"))

N = 100000
E = 3200000
IN_CH = 512
OUT_CH = 64
NCORES = 8
PER = N // NCORES            # 12500 rows per core
RPAD = 12544                 # rows contributed per core (12500 + 44 zeros)
NB = 4                       # src buckets (2 cores each)
REGION = 2 * RPAD            # 25088 rows per bucket region in g_all
ZROW = PER                   # bucket-local index of a guaranteed zero row
NSTRIP = RPAD // 128         # 98 strips of 128 dsts
OCTW = 8                     # strips per octet
NTILE = PER // 128           # 97.65... -> careful: 12500/128 not integer!

assert RPAD % 128 == 0


def _bf16(a):
    import ml_dtypes
    return np.asarray(a, dtype=np.float32).astype(ml_dtypes.bfloat16)


def _wrap_idx(flat):
    """int16 ucode layout: idx i -> (partition i%16, col i//16), replicated
    to all 8 gpsimd cores (16-partition groups)."""
    n = flat.shape[0]
    assert n % 16 == 0
    out = np.zeros((128, n // 16), dtype=np.int16)
    w = flat.reshape(n // 16, 16).T
    for r in range(8):
        out[r * 16:(r + 1) * 16, :] = w
    return out


def _preprocess(x, edge_index, W, b, alpha):
    """Host-side index plumbing + sharding. Returns (in_maps, meta)."""
    src_g = np.asarray(edge_index[0])
    dst_g = np.asarray(edge_index[1])
    x = np.asarray(x, dtype=np.float32)
    W = np.asarray(W, dtype=np.float32)
    b = np.asarray(b, dtype=np.float32)
    alpha = np.asarray(alpha, dtype=np.float32)

    deg = np.bincount(dst_g, minlength=N).astype(np.int64) + 1  # incl self loop

    owner = dst_g // PER
    # bucket-region-local row index of every global node id
    s_core = np.arange(N) // PER
    g_row_local = (RPAD * (s_core % 2) + (np.arange(N) % PER)).astype(np.int64)

    per_core = []
    for c in range(NCORES):
        m = owner == c
        s = src_g[m].astype(np.int64)
        d = (dst_g[m] - c * PER).astype(np.int64)
        bkt = s // (2 * PER)
        loc = g_row_local[s]  # bucket-local row of src in g_all region
        per_core.append((s, d, bkt, loc))

    # per (core, dst, bucket) counts
    cnts = np.zeros((NCORES, RPAD, NB), dtype=np.int32)
    orders = []  # per core: edge order sorted by (bucket, dst)
    offs = []
    for c in range(NCORES):
        s, d, bkt, loc = per_core[c]
        key = bkt * RPAD + d
        cnt = np.bincount(key, minlength=NB * RPAD).reshape(NB, RPAD)
        cnts[c] = cnt.T
        order = np.argsort(key, kind="stable")
        off = np.zeros(NB * RPAD + 1, dtype=np.int64)
        np.cumsum(cnt.reshape(-1), out=off[1:])
        orders.append(order)
        offs.append(off)

    # per-bucket dst ordering (descending count), shared strip/octet structure
    sigma = np.zeros((NCORES, NB, RPAD), dtype=np.int64)
    for c in range(NCORES):
        for bb in range(NB):
            sigma[c, bb] = np.argsort(-cnts[c, :, bb], kind="stable")

    # octets: groups of strips; unit (b, oct) has uniform J = max count over
    # ALL cores (SPMD: one program)
    octs = []  # list of (s0, w) strip ranges
    s0 = 0
    while s0 < NSTRIP:
        w = min(OCTW, NSTRIP - s0)
        octs.append((s0, w))
        s0 += w

    J_units = []
    for bb in range(NB):
        for (s0, w) in octs:
            jmax = 1
            for c in range(NCORES):
                dset = sigma[c, bb, s0 * 128:(s0 + w) * 128]
                jmax = max(jmax, int(cnts[c, dset, bb].max()))
            J_units.append(jmax)

    # build gather idx tables per core: one big int16 tensor per core
    # [128, total_cols] where unit (b,oct) occupies cols [unit_off, +64*J*? ]
    # each gather j covers 128*w*? slots... slots per j = 128*w*16? cols=slots/16
    unit_meta = []  # (b, s0, w, J, col_off)
    col_off = 0
    ui = 0
    for bb in range(NB):
        for (s0, w) in octs:
            J = J_units[ui]
            ui += 1
            ncols = (128 * w * J) // 16
            unit_meta.append((bb, s0, w, J, col_off))
            col_off += ncols
    total_cols = col_off

    gidx = np.zeros((NCORES, 128, total_cols), dtype=np.int16)
    pad_slots_total = 0
    for c in range(NCORES):
        s, d, bkt, loc = per_core[c]
        order = orders[c]
        off = offs[c]
        loc_sorted = loc[order]
        for (bb, s0, w, J, co) in unit_meta:
            dset = sigma[c, bb, s0 * 128:(s0 + w) * 128]  # [w*128] dst ids
            cnt_u = cnts[c, dset, bb]  # [w*128]
            base = off[bb * RPAD + dset]  # [w*128] offsets into loc_sorted
            # slots: for j in range(J): for t in range(w): for p in range(128)
            jj = np.arange(J)[:, None]  # [J, 1]
            valid = jj < cnt_u[None, :]  # [J, w*128]
            eidx = base[None, :] + np.minimum(jj, np.maximum(cnt_u[None, :] - 1, 0))
            eidx = np.minimum(eidx, loc_sorted.shape[0] - 1)
            # Padded slots read a zero row. Spread them over the 2*44
            # distinct zero rows of the region (rows PER..RPAD of both
            # core blocks): identical-address reads serialize the SDMA
            # read pipeline (~400ns/desc instead of ~25ns).
            pos = np.arange(J * w * 128).reshape(J, w * 128)
            zvar = (ZROW + pos % 44
                    + RPAD * ((pos // 44) % 2)).astype(np.int64)
            vals = np.where(valid, loc_sorted[eidx], zvar).astype(np.int16)
            # order within unit: j-major, then slot i = t*128+p
            flat = vals.reshape(J * w * 128)
            pad_slots_total += int((~valid).sum())
            gidx[c, :, co:co + flat.size // 16] = _wrap_idx(flat)

    # P4 combine index tables: 5 gathers x 12544 idx
    # out slot i -> (p=i%128, col=i//128); dst d(i) = ?  we need out tile
    # [128, 98, 64] with (P, S) = dst P*98+S. gather writes slot i to
    # (i%128, i//128) -> so idx position i corresponds to dst (i%128)*98+(i//128)
    ii = np.arange(RPAD)
    dd = (ii % 128) * NSTRIP + (ii // 128)  # dst id at idx position i
    cidx = np.zeros((NCORES, 5, 128, RPAD // 16), dtype=np.int16)
    for c in range(NCORES):
        for bb in range(NB):
            # dst d -> sigma position -> partial_b flat row (p*98 + s)
            spos = np.empty(RPAD, dtype=np.int64)
            spos[sigma[c, bb]] = np.arange(RPAD)
            rows = (spos[dd] % 128) * NSTRIP + (spos[dd] // 128)
            cidx[c, bb] = _wrap_idx(rows.astype(np.int16))
        # self-loop: own g rows, bucket-local within region c//2
        rows_own = np.where(dd < PER, dd, ZROW)
        cidx[c, 4] = _wrap_idx(rows_own.astype(np.int16))

    # x blocks bf16, contiguous per tile: xTb[c][r][p][k*128+m]
    #   = x[c*PER + 128r + m, 128k + p]
    NT = RPAD // 128  # 98 row tiles
    xTb = np.zeros((NCORES, NT, 128, 4 * 128), dtype=np.float32)
    for c in range(NCORES):
        xr = x[c * PER:(c + 1) * PER]  # [12500, 512]
        xp = np.zeros((RPAD, IN_CH), dtype=np.float32)
        xp[:PER] = xr
        blocks = xp.reshape(NT, 128, 4, 128)  # [r, m, k, p]
        xTb[c] = blocks.transpose(0, 3, 2, 1).reshape(NT, 128, 4 * 128)
    xTb = _bf16(xTb)

    # W packed [p, k*64+c] = W[128k+p, c]
    Wb = _bf16(W.reshape(4, 128, OUT_CH).transpose(1, 0, 2).reshape(128, 4 * OUT_CH))

    # degree tables
    deg_p1 = np.zeros((NCORES, 128, NT), dtype=np.float32)
    deg_p4 = np.zeros((NCORES, 128, NSTRIP), dtype=np.float32)
    for c in range(NCORES):
        degc = np.ones(RPAD, dtype=np.float32)
        degc[:PER] = deg[c * PER:(c + 1) * PER]
        deg_p1[c] = degc.reshape(NT, 128).T  # (p, r) = row 128r+p
        deg_p4[c] = degc.reshape(128, NSTRIP)  # (p, s) = d = p*98+s

    bias_rep = np.tile(b[None, :], (128, 1)).astype(np.float32)
    alpha_rep = np.tile(alpha[None, :], (128, 1)).astype(np.float32)

    in_maps = []
    for c in range(NCORES):
        in_maps.append({
            "xTb": xTb[c],
            "Wb": Wb,
            "gidx": gidx[c],
            "cidx": cidx[c],
            "deg_p1": deg_p1[c],
            "deg_p4": deg_p4[c],
            "bias_rep": bias_rep,
            "alpha_rep": alpha_rep,
        })
    meta = {"unit_meta": unit_meta, "total_cols": total_cols,
            "pad_frac": pad_slots_total / max(1, sum(
                128 * w * J for (_, _, w, J, _) in unit_meta) * NCORES)}
    return in_maps, meta


def _patch_walrus_flags():
    import concourse.bass_utils as bu
    if getattr(bu, "_scratch_patched", False):
        return
    orig = bu.get_walrus_args

    def patched(arch, tmpdir, *, dve_root=None):
        args = orig(arch, tmpdir, dve_root=dve_root)
        import os as _os
        sz = _os.environ.get("K_DMA_SCRATCH")
        if sz:
            args = [f"--dynamic-dma-scratch-size-per-partition={sz}", *args]
        return args

    bu.get_walrus_args = patched
    bu._scratch_patched = True


def _build_program(unit_meta, total_cols):
    import concourse.bacc as bacc
    import concourse.mybir as mybir
    import concourse.tile as tile
    from concourse.masks import make_identity

    f32 = mybir.dt.float32
    f32r = mybir.dt.float32r
    bf16 = mybir.dt.bfloat16
    i16 = mybir.dt.int16
    NT = RPAD // 128

    _patch_walrus_flags()
    nc = bacc.Bacc("TRN2", target_bir_lowering=False, debug=False,
                   num_devices=NCORES, num_swdge_queues=4,
                   dynamic_dma_scratch_size=int(os.environ.get("K_SCRATCH", "16384")))

    xTb_t = nc.dram_tensor("xTb", [NT, 128, 4 * 128], bf16, kind="ExternalInput")
    Wb_t = nc.dram_tensor("Wb", [128, 4 * OUT_CH], bf16, kind="ExternalInput")
    gidx_t = nc.dram_tensor("gidx", [128, total_cols], i16, kind="ExternalInput")
    cidx_t = nc.dram_tensor("cidx", [5, 128, RPAD // 16], i16, kind="ExternalInput")
    degp1_t = nc.dram_tensor("deg_p1", [128, NT], f32, kind="ExternalInput")
    degp4_t = nc.dram_tensor("deg_p4", [128, NSTRIP], f32, kind="ExternalInput")
    bias_t = nc.dram_tensor("bias_rep", [128, OUT_CH], f32, kind="ExternalInput")
    alpha_t = nc.dram_tensor("alpha_rep", [128, OUT_CH], f32, kind="ExternalInput")
    out_t = nc.dram_tensor("out", [RPAD, OUT_CH], f32, kind="ExternalOutput")

    with tile.TileContext(nc) as tc:
        with (
            tc.tile_pool(name="dram", bufs=1, space="DRAM") as dram,
            tc.tile_pool(name="const", bufs=1) as cpool,
            tc.tile_pool(name="p1", bufs=3) as p1pool,
            tc.tile_pool(name="gath", bufs=2) as gpool,
            tc.tile_pool(name="evac", bufs=3) as epool,
            tc.tile_pool(name="p4", bufs=1) as p4pool,
            tc.tile_pool(name="psum_h", bufs=2, space="PSUM") as ppool_h,
            tc.tile_pool(name="psum_a", bufs=4, space="PSUM") as ppool,
        ):
            g_in = dram.tile([RPAD, OUT_CH], f32r)
            g_all = dram.tile([NCORES * RPAD, OUT_CH], f32r)
            partials = [dram.tile([RPAD, OUT_CH], f32r, tag=f"part{_b}", name=f"part{_b}")
                        for _b in range(NB)]

            # ---------------- P1: h = x @ W, g = dinv * h -----------------
            Wsb = cpool.tile([128, 4, OUT_CH], bf16)
            nc.sync.dma_start(Wsb[:], Wb_t[:].rearrange("p (k c) -> p k c", k=4))
            degp1 = cpool.tile([128, NT], f32)
            nc.sync.dma_start(degp1[:], degp1_t[:])
            dinv1 = cpool.tile([128, NT], f32)
            nc.scalar.activation(dinv1[:], degp1[:],
                                 mybir.ActivationFunctionType.Sqrt)
            nc.vector.reciprocal(dinv1[:], dinv1[:])

            for r in range(NT):
                xt = p1pool.tile([128, 4, 128], bf16)
                nc.sync.dma_start(
                    xt[:], xTb_t[r].rearrange("p (k m) -> p k m", k=4))
                hp = ppool_h.tile([128, OUT_CH], f32, space="PSUM", tag="hp")
                for k in range(4):
                    nc.tensor.matmul(hp[:], xt[:, k], Wsb[:, k],
                                     start=(k == 0), stop=(k == 3))
                grow = p1pool.tile([128, OUT_CH], f32r, tag="grow")
                nc.scalar.activation(grow[:], hp[:],
                                     mybir.ActivationFunctionType.Copy,
                                     scale=dinv1[:, r:r + 1])
                nc.sync.dma_start(g_in[r * 128:(r + 1) * 128, :], grow[:])

            # ---------------- P2: AllGather ------------------------------
            nc.gpsimd.collective_compute(
                "AllGather", mybir.AluOpType.bypass,
                replica_groups=[list(range(NCORES))],
                ins=[g_in.opt()], outs=[g_all.opt()],
            )

            # ---------------- P3: gather + segmented reduce --------------
            ident_raw = cpool.tile([128, 128], f32)
            make_identity(nc, ident_raw[:])
            ident_f = cpool.tile([128, 128], f32r)
            nc.scalar.activation(ident_f[:], ident_raw[:],
                                 mybir.ActivationFunctionType.Copy)

            # P4 constants + accumulators, loaded early; per-bucket combine
            # gathers are interleaved into the P3 stream as soon as each
            # bucket's partials are complete.
            cidx_sbs = []
            for _g in range(5):
                cs = cpool.tile([128, RPAD // 16], i16, name=f"cidx{_g}",
                                tag=f"cidx{_g}")
                nc.sync.dma_start(cs[:], cidx_t[_g])
                cidx_sbs.append(cs)
            degp4 = cpool.tile([128, NSTRIP], f32)
            nc.sync.dma_start(degp4[:], degp4_t[:])
            dinv4 = cpool.tile([128, NSTRIP], f32)
            nc.scalar.activation(dinv4[:], degp4[:],
                                 mybir.ActivationFunctionType.Sqrt)
            nc.vector.reciprocal(dinv4[:], dinv4[:])
            bias_sb = cpool.tile([128, OUT_CH], f32)
            nc.sync.dma_start(bias_sb[:], bias_t[:])
            alpha_sb = cpool.tile([128, OUT_CH], f32)
            nc.sync.dma_start(alpha_sb[:], alpha_t[:])
            accs = p4pool.tile([128, NSTRIP, OUT_CH], f32, tag="c_acc")
            tmp = p4pool.tile([128, NSTRIP, OUT_CH], f32, tag="c_tmp")

            def emit_p4(tgt, src_ap, ci):
                for k in range(0, NSTRIP, 8):
                    kw = min(8, NSTRIP - k)
                    nc.gpsimd.dma_gather(
                        tgt[:, k:k + kw], src_ap,
                        cidx_sbs[ci][:, 8 * k:8 * k + 8 * kw],
                        128 * kw, 128 * kw, OUT_CH,
                        single_packet=SP, queue_num=(k // 8) % 4)

            def finish_bucket(b):
                if b == 0:
                    emit_p4(accs, partials[0][:].bitcast(f32), 0)
                    emit_p4(tmp, g_in[:].bitcast(f32), 4)
                else:
                    emit_p4(tmp, partials[b][:].bitcast(f32), b)
                nc.vector.tensor_add(accs[:], accs[:], tmp[:])

            prev_bb = 0
            for (bb, s0, w, J, co) in unit_meta:
                if bb != prev_bb:
                    finish_bucket(prev_bb)
                    prev_bb = bb
                nidx = 128 * w
                ncols_j = nidx // 16
                idx_sb = gpool.tile([128, J, ncols_j], i16, tag="idx")
                nc.sync.dma_start(
                    idx_sb[:],
                    gidx_t[:, co:co + J * ncols_j].rearrange(
                        "p (j n) -> p j n", j=J))
                acc = ppool.tile([128, w * OUT_CH], f32, space="PSUM", tag="acc")
                for jg in range(0, J, JCAP):
                    jgn = min(JCAP, J - jg)
                    msgs = gpool.tile([128, jgn, w, OUT_CH], f32r, tag="msgs")
                    for j0 in range(0, jgn, JCHUNK):
                        jn = min(JCHUNK, jgn - j0)
                        nc.gpsimd.dma_gather(
                            msgs[:, j0:j0 + jn].rearrange(
                                "p j t c -> p (j t) c"),
                            g_all[REGION * bb:REGION * (bb + 1), :],
                            idx_sb[:, jg + j0:jg + j0 + jn].rearrange(
                                "p j n -> p (j n)"),
                            nidx * jn, nidx * jn, OUT_CH,
                            single_packet=SP,
                            queue_num=((jg + j0) // JCHUNK) % 4,
                        )
                    for j in range(jgn):
                        nc.tensor.matmul(
                            acc[:], ident_f[:],
                            msgs[:, j].rearrange("p t c -> p (t c)"),
                            start=(jg + j == 0), stop=(jg + j == J - 1))
                ev = epool.tile([128, w * OUT_CH], f32r, tag="ev")
                nc.vector.tensor_copy(ev[:], acc[:])
                nc.sync.dma_start(
                    partials[bb][:].rearrange(
                        "(p s) c -> p s c", p=128)[:, s0:s0 + w, :],
                    ev[:].rearrange("p (t c) -> p t c", c=OUT_CH))

            finish_bucket(3)

            # ---------------- P4 tail: pointwise -------------------------
            # z = accs * dinv4 + bias (in place in accs)
            nc.vector.tensor_tensor(
                out=accs[:], in0=accs[:],
                in1=dinv4[:].rearrange("p (s c) -> p s c", c=1).to_broadcast(
                    [128, NSTRIP, OUT_CH]),
                op=mybir.AluOpType.mult)
            nc.vector.tensor_tensor(
                out=accs[:], in0=accs[:],
                in1=bias_sb[:].rearrange("p (s c) -> p s c", s=1).to_broadcast(
                    [128, NSTRIP, OUT_CH]),
                op=mybir.AluOpType.add)
            # tmp = relu(z); accs = (z - relu(z)) * alpha + relu(z)
            nc.scalar.activation(tmp[:], accs[:],
                                 mybir.ActivationFunctionType.Relu)
            nc.vector.tensor_tensor(out=accs[:], in0=accs[:], in1=tmp[:],
                                    op=mybir.AluOpType.subtract)
            nc.vector.tensor_tensor(
                out=accs[:], in0=accs[:],
                in1=alpha_sb[:].rearrange("p (s c) -> p s c", s=1).to_broadcast(
                    [128, NSTRIP, OUT_CH]),
                op=mybir.AluOpType.mult)
            nc.vector.tensor_add(accs[:], accs[:], tmp[:])
            nc.sync.dma_start(
                out_t[:].rearrange("(p s) c -> p s c", p=128), accs[:])

    nc.compile()
    return nc


LAST_EXEC_NS = None
TRACE = False


def kernel(x, edge_index, W, b, alpha):
    global LAST_EXEC_NS
    from concourse.bass_utils import run_bass_kernel_spmd

    in_maps, meta = _preprocess(x, edge_index, W, b, alpha)
    print("pad_frac:", meta["pad_frac"], "total_cols:", meta["total_cols"])
    nc = _build_program(meta["unit_meta"], meta["total_cols"])
    res = run_bass_kernel_spmd(nc, in_maps, core_ids=list(range(NCORES)),
                               trace=TRACE)
    LAST_EXEC_NS = res.exec_time_ns
    if TRACE:
        print("scope times:", res.per_core_scope_times)
        print("trace:", res.instructions_and_trace[1]
              if res.instructions_and_trace else None)
    out = np.concatenate([r["out"][:PER] for r in res.results], axis=0)
    return out


if __name__ == "__main__":
    pass

